# revision 2
# baseline (speedup 1.0000x reference)
"""Trainium2 Bass kernel v2 for nn_Decoder_35527969472565.

Same algorithmic reduction as v1 (attention softmax is shift-invariant =>
context is step-independent), but the matmul path runs in bf16 (1 cycle/row
on the PE vs 4 for fp32) and the per-step gate input is produced by a single
K=112 matmul against an augmented x operand:

    g_init[b, :] = x_t @ Wihx + C0[b, :]
                 = [x_t ; I32] . [Wihx ; C0]     (K = 80 + 32 = 112)

where the host bakes the 32-identity pattern under the x columns, so neither
a DRAM roundtrip for the precomputed gate inputs nor a separate bias matmul
is needed. The C0 rows of the RHS slab are written on-device (C0 depends on
the attention context). PSUM accumulation and the c-state stay fp32.
"""

import numpy as np

import concourse.bacc as bacc
import concourse.mybir as mybir
import concourse.tile as tile

F32 = mybir.dt.float32
BF = mybir.dt.bfloat16
AF = mybir.ActivationFunctionType

B = 32
S = 200
E2 = 512
H = 1024
M = 80
NG = 8          # gate groups (512 cols each)
GW = 512        # group width
KC = 8          # K chunks of 128 over H
G4 = 4 * H
KA = 128        # augmented K: x rows 0:80, zeros, C0 rows 96:128


def gate_perm():
    P = []
    for n in range(NG):
        P += list(range(0 * H + n * 128, 0 * H + (n + 1) * 128))   # i
        P += list(range(1 * H + n * 128, 1 * H + (n + 1) * 128))   # f
        P += list(range(3 * H + n * 128, 3 * H + (n + 1) * 128))   # o
        P += list(range(2 * H + n * 128, 2 * H + (n + 1) * 128))   # g
    return np.array(P)


def prep_inputs(inp, T):
    import ml_dtypes
    P = gate_perm()
    f32 = np.float32
    bf = ml_dtypes.bfloat16
    Wih0 = np.asarray(inp["Wih0"], f32)[P]
    Whh0 = np.asarray(inp["Whh0"], f32)[P]
    Wih1 = np.asarray(inp["Wih1"], f32)[P]
    Whh1 = np.asarray(inp["Whh1"], f32)[P]
    b0 = (np.asarray(inp["bih0"], f32) + np.asarray(inp["bhh0"], f32))[P]
    b1 = (np.asarray(inp["bih1"], f32) + np.asarray(inp["bhh1"], f32))[P]
    enc = np.asarray(inp["encoder_outputs"], f32)
    x = np.asarray(inp["audio_targets"], f32)[:, :T, :]

    d = {}
    d["WhhT0"] = np.ascontiguousarray(Whh0.T).astype(bf)
    d["WhhT1"] = np.ascontiguousarray(Whh1.T).astype(bf)
    d["WihT1"] = np.ascontiguousarray(Wih1.T).astype(bf)
    d["WihxT"] = np.ascontiguousarray(Wih0[:, :M].T).astype(bf)
    d["WiheT"] = np.ascontiguousarray(Wih0[:, M:].T).astype(bf)
    d["WfcT"] = np.ascontiguousarray(np.asarray(inp["Wfc"], f32).T).astype(bf)
    d["b0row"] = b0.reshape(1, G4).astype(bf)
    d["b1row"] = b1.reshape(1, G4).astype(bf)
    d["bfcrow"] = np.asarray(inp["bfc"], f32).reshape(1, M).astype(bf)
    d["enc"] = np.ascontiguousarray(enc).astype(bf)
    d["waeRep"] = np.tile(np.asarray(inp["Wa"], f32)[0:1, H:], (128, 1))
    d["w0col"] = np.full((S, 1), 1.0 / S, f32).astype(bf)
    # augmented x operand: rows 0:80 = x^T (col = t*B + b), rows 80:112 a
    # 32-identity pattern per b so the C0 rows of the RHS slab broadcast in.
    xT = np.ascontiguousarray(x.transpose(2, 1, 0).reshape(M, T * B))
    xaug = np.zeros((KA, T * B), f32)
    xaug[:M] = xT
    idx = np.arange(T * B)
    xaug[96 + (idx % B), idx] = 1.0
    xaug[96:, 0:B] = 0.0         # t=0 uses C0init via an extra matmul
    d["xaug"] = xaug.astype(bf)
    d["i32"] = np.eye(B, dtype=f32).astype(bf)
    d["ones128"] = np.ones((1, 128), f32).astype(bf)
    return d


def build(T=400):
    nc = bacc.Bacc()
    shapes = {"WhhT0": ((H, G4), BF), "WhhT1": ((H, G4), BF),
              "WihT1": ((H, G4), BF), "WihxT": ((M, G4), BF),
              "WiheT": ((E2, G4), BF), "WfcT": ((H, M), BF),
              "b0row": ((1, G4), BF), "b1row": ((1, G4), BF),
              "bfcrow": ((1, M), BF), "enc": ((B, S, E2), BF),
              "waeRep": ((128, E2), F32), "w0col": ((S, 1), BF),
              "xaug": ((KA, T * B), BF), "i32": ((B, B), BF),
              "ones128": ((1, 128), BF)}
    names = list(shapes)
    t_in = {n: nc.dram_tensor(n, list(shapes[n][0]), shapes[n][1],
                              kind="ExternalInput")
            for n in names}
    out = nc.dram_tensor("out", [B, T, M], F32, kind="ExternalOutput")

    H0T = nc.dram_tensor("H0T", [T, 128, 256], BF)
    H1T = nc.dram_tensor("H1T", [T, 128, 256], BF)
    Q1 = nc.dram_tensor("Q1", [T * B, G4], BF)

    with tile.TileContext(nc) as tc:
        with (
            tc.tile_pool(name="wp", bufs=1) as wp,
            tc.tile_pool(name="sb", bufs=2) as sb,
            tc.tile_pool(name="sb3", bufs=3) as sb3,
            tc.tile_pool(name="psg", bufs=3, space="PSUM") as psg,
            tc.tile_pool(name="psb", bufs=1, space="PSUM") as psb,
        ):
            # ---------- resident small tensors ----------
            i32b = wp.tile([B, B], BF, tag="i32b")
            nc.sync.dma_start(i32b[:], t_in["i32"][:])
            ones128b = wp.tile([1, 128], BF, tag="ones128b")
            nc.sync.dma_start(ones128b[:], t_in["ones128"][:])
            waer = wp.tile([128, E2], F32, tag="waer")
            nc.sync.dma_start(waer[:], t_in["waeRep"][:])
            w0a = wp.tile([128, 1], BF, tag="w0a")
            nc.sync.dma_start(w0a[:], t_in["w0col"][0:128, :])
            w0c = wp.tile([72, 1], BF, tag="w0c")
            nc.sync.dma_start(w0c[:], t_in["w0col"][128:200, :])
            b0b = wp.tile([1, G4], BF, tag="b0b")
            nc.sync.dma_start(b0b[:], t_in["b0row"][:])
            b1b = wp.tile([1, G4], BF, tag="b1b")
            nc.sync.dma_start(b1b[:], t_in["b1row"][:])
            bfcb = wp.tile([1, M], BF, tag="bfcb")
            nc.sync.dma_start(bfcb[:], t_in["bfcrow"][:])

            # ---------- attention pass 1: scores + uniform-weight ctx ----------
            scT_a = wp.tile([128, B], F32, tag="scT_a")
            scT_c = wp.tile([96, B], F32, tag="scT_c")
            nc.vector.memset(scT_c[:], 0.0)
            cps0 = [psb.tile([128, B], F32, tag=f"cb{hs}", name=f"cps0_{hs}")
                    for hs in range(4)]
            for b in range(B):
                ea = sb.tile([128, E2], BF, tag="ea", name=f"ea{b}")
                nc.sync.dma_start(ea[:], t_in["enc"][b, 0:128, :])
                ec = sb.tile([72, E2], BF, tag="ec", name=f"ec{b}")
                nc.sync.dma_start(ec[:], t_in["enc"][b, 128:200, :])
                tha = sb.tile([128, E2], F32, tag="tha")
                nc.scalar.activation(tha[:], ea[:], AF.Tanh)
                thc = sb.tile([72, E2], F32, tag="thc")
                nc.scalar.activation(thc[:], ec[:], AF.Tanh)
                pra = sb.tile([128, E2], F32, tag="pra")
                nc.vector.tensor_mul(out=pra[:], in0=tha[:], in1=waer[:])
                prc = sb.tile([72, E2], F32, tag="prc")
                nc.vector.tensor_mul(out=prc[:], in0=thc[:], in1=waer[0:72, :])
                nc.vector.reduce_sum(scT_a[:, b:b + 1], pra[:],
                                     axis=mybir.AxisListType.X)
                nc.vector.reduce_sum(scT_c[0:72, b:b + 1], prc[:],
                                     axis=mybir.AxisListType.X)
                for hs in range(4):
                    nc.tensor.matmul(cps0[hs][:, b:b + 1],
                                     ea[:, 128 * hs:128 * (hs + 1)], w0a[:],
                                     start=True, stop=False)
                    nc.tensor.matmul(cps0[hs][:, b:b + 1],
                                     ec[:, 128 * hs:128 * (hs + 1)], w0c[:],
                                     start=False, stop=True)
            ctx0T = []
            for hs in range(4):
                ct = wp.tile([128, B], BF, tag=f"c0T{hs}")
                nc.scalar.activation(ct[:], cps0[hs][:], AF.Copy)
                ctx0T.append(ct)

            # ---------- softmax over scores ----------
            score = wp.tile([B, 224], F32, tag="score")
            for j in range(4):
                nc.vector.transpose(score[:, 32 * j:32 * (j + 1)],
                                    scT_a[32 * j:32 * (j + 1), :])
            for j in range(3):
                nc.vector.transpose(score[:, 128 + 32 * j:160 + 32 * j],
                                    scT_c[32 * j:32 * (j + 1), :])
            mx = sb.tile([B, 1], F32, tag="mx")
            nc.vector.reduce_max(mx[:], score[:, 0:S], axis=mybir.AxisListType.X)
            nmx = sb.tile([B, 1], F32, tag="nmx")
            nc.vector.tensor_scalar_mul(nmx[:], mx[:], -1.0)
            ew = wp.tile([B, 224], F32, tag="ew")
            nc.vector.memset(ew[:], 0.0)
            nc.scalar.activation(ew[:, 0:S], score[:, 0:S], AF.Exp, bias=nmx[:])
            sm = sb.tile([B, 1], F32, tag="sm")
            nc.vector.reduce_sum(sm[:], ew[:, 0:S], axis=mybir.AxisListType.X)
            rs = sb.tile([B, 1], F32, tag="rs")
            nc.vector.reciprocal(rs[:], sm[:])
            wgt = wp.tile([B, 224], F32, tag="wgt")
            nc.vector.tensor_scalar_mul(wgt[:], ew[:], rs[:])
            wT_a = wp.tile([128, B], F32, tag="wT_a")
            wT_c = wp.tile([96, B], F32, tag="wT_c")
            for j in range(4):
                nc.vector.transpose(wT_a[32 * j:32 * (j + 1), :],
                                    wgt[:, 32 * j:32 * (j + 1)])
            for j in range(3):
                nc.vector.transpose(wT_c[32 * j:32 * (j + 1), :],
                                    wgt[:, 128 + 32 * j:128 + 32 * (j + 1)])
            wTab = wp.tile([128, B], BF, tag="wTab")
            nc.scalar.activation(wTab[:], wT_a[:], AF.Copy)
            wTcb = wp.tile([96, B], BF, tag="wTcb")
            nc.scalar.activation(wTcb[:], wT_c[:], AF.Copy)

            # ---------- attention pass 2: softmax-weighted ctx ----------
            cps = [psb.tile([128, B], F32, tag=f"cb{hs}", name=f"cps_{hs}")
                   for hs in range(4)]
            for b in range(B):
                ea = sb.tile([128, E2], BF, tag="ea", name=f"cea{b}")
                nc.sync.dma_start(ea[:], t_in["enc"][b, 0:128, :])
                ec = sb.tile([72, E2], BF, tag="ec", name=f"cec{b}")
                nc.sync.dma_start(ec[:], t_in["enc"][b, 128:200, :])
                for hs in range(4):
                    nc.tensor.matmul(cps[hs][:, b:b + 1],
                                     ea[:, 128 * hs:128 * (hs + 1)],
                                     wTab[:, b:b + 1], start=True, stop=False)
                    nc.tensor.matmul(cps[hs][:, b:b + 1],
                                     ec[:, 128 * hs:128 * (hs + 1)],
                                     wTcb[0:72, b:b + 1], start=False, stop=True)
            ctxT = []
            for hs in range(4):
                ct = wp.tile([128, B], BF, tag=f"cT{hs}")
                nc.scalar.activation(ct[:], cps[hs][:], AF.Copy)
                ctxT.append(ct)

            # ---------- C0 = ctx @ Wihe + b0 (run into wx slab, init to tile) ----------
            wxslab = wp.tile([KA, G4], BF, tag="wxslab")
            nc.vector.memset(wxslab[:], 0.0)
            nc.sync.dma_start(wxslab[0:M, :], t_in["WihxT"][:])
            c0initb = wp.tile([B, G4], BF, tag="c0initb")
            wihe = wp.tile([128, 4 * G4], BF, tag="bigW")
            nc.sync.dma_start(wihe[:],
                              t_in["WiheT"].ap().rearrange("(c p) n -> p c n", p=128))
            for idx, ctx_t in ((0, ctxT), (1, ctx0T)):
                for n in range(NG):
                    cq = psg.tile([B, GW], F32, tag="g")
                    nc.tensor.matmul(cq[:], ones128b[:, 0:B],
                                     b0b[:, GW * n:GW * (n + 1)],
                                     start=True, stop=False)
                    for hs in range(4):
                        nc.tensor.matmul(
                            cq[:], ctx_t[hs][:],
                            wihe[:, G4 * hs + GW * n:G4 * hs + GW * (n + 1)],
                            start=False, stop=(hs == 3))
                    if idx == 0:
                        nc.scalar.activation(
                            wxslab[96:128, GW * n:GW * (n + 1)], cq[:], AF.Copy)
                    else:
                        nc.scalar.activation(
                            c0initb[:, GW * n:GW * (n + 1)], cq[:], AF.Copy)

            # xaug slab resident for layer-0
            xslab = wp.tile([KA, T * B], BF, tag="xslab")
            nc.sync.dma_start(xslab[:], t_in["xaug"][:])

            # ---------- shared LSTM cell elementwise ----------
            def cell_elem(g_ps, cprev, hT_next, n):
                sg = sb3.tile([B, 384], F32, tag="sg")
                nc.scalar.activation(sg[:], g_ps[:, 0:384], AF.Sigmoid)
                tg = sb3.tile([B, 128], F32, tag="tg")
                nc.scalar.activation(tg[:], g_ps[:, 384:512], AF.Tanh)
                t2 = sb.tile([B, 128], F32, tag="t2")
                nc.vector.tensor_mul(out=t2[:], in0=sg[:, 0:128], in1=tg[:])
                cnew = sb.tile([B, 128], F32, tag=f"c{n}")
                if cprev is None:
                    nc.vector.tensor_copy(cnew[:], t2[:])
                else:
                    t1 = sb.tile([B, 128], F32, tag="t1")
                    nc.vector.tensor_mul(out=t1[:], in0=sg[:, 128:256], in1=cprev[:])
                    nc.vector.tensor_add(out=cnew[:], in0=t1[:], in1=t2[:])
                tc2 = sb.tile([B, 128], F32, tag="tc2")
                nc.scalar.activation(tc2[:], cnew[:], AF.Tanh)
                hn = sb.tile([B, 128], BF, tag="hn")
                nc.vector.tensor_mul(out=hn[:], in0=sg[:, 256:384], in1=tc2[:])
                for j in range(4):
                    nc.vector.transpose(
                        hT_next[32 * j:32 * (j + 1), 32 * n:32 * (n + 1)],
                        hn[:, 32 * j:32 * (j + 1)])
                return cnew

            # ---------- phase A: layer-0 recurrence ----------
            big = wp.tile([128, KC * G4], BF, tag="bigW")
            nc.sync.dma_start(big[:],
                              t_in["WhhT0"].ap().rearrange("(c p) n -> p c n", p=128))
            hT = None
            cst = [None] * NG
            for t in range(T):
                hT_next = sb.tile([128, 256], BF, tag="hTn")
                for n in range(NG):
                    g_ps = psg.tile([B, GW], F32, tag="g")
                    nc.tensor.matmul(g_ps[:], xslab[:, B * t:B * (t + 1)],
                                     wxslab[:, GW * n:GW * (n + 1)],
                                     start=True, stop=False)
                    if t == 0:
                        nc.tensor.matmul(g_ps[:], i32b[:],
                                         c0initb[:, GW * n:GW * (n + 1)],
                                         start=False, stop=(hT is None))
                    if hT is not None:
                        for c in range(KC):
                            nc.tensor.matmul(
                                g_ps[:], hT[:, 32 * c:32 * (c + 1)],
                                big[:, G4 * c + GW * n:G4 * c + GW * (n + 1)],
                                start=False, stop=(c == KC - 1))
                    cst[n] = cell_elem(g_ps, cst[n], hT_next, n)
                nc.scalar.dma_start(H0T[t], hT_next[:])
                hT = hT_next

            # ---------- mid: Q1 = H0 @ Wih1T + b1 ----------
            big = wp.tile([128, KC * G4], BF, tag="bigW")
            nc.sync.dma_start(big[:],
                              t_in["WihT1"].ap().rearrange("(c p) n -> p c n", p=128))
            assert T % 4 == 0
            for m in range(T // 4):
                slab = sb3.tile([128, 1024], BF, tag="mslab", name=f"mslab{m}")
                nc.sync.dma_start(
                    slab[:],
                    H0T.ap()[4 * m:4 * (m + 1)].rearrange(
                        "t p (c b) -> p c t b", c=8))
                for n in range(NG):
                    qps = psg.tile([128, GW], F32, tag="g")
                    nc.tensor.matmul(qps[:], ones128b[:],
                                     b1b[:, GW * n:GW * (n + 1)],
                                     start=True, stop=False)
                    for c in range(KC):
                        nc.tensor.matmul(
                            qps[:], slab[:, 128 * c:128 * (c + 1)],
                            big[:, G4 * c + GW * n:G4 * c + GW * (n + 1)],
                            start=False, stop=(c == KC - 1))
                    qsb = sb3.tile([128, GW], BF, tag="qsb")
                    nc.scalar.activation(qsb[:], qps[:], AF.Copy)
                    nc.sync.dma_start(
                        Q1[128 * m:128 * (m + 1), GW * n:GW * (n + 1)], qsb[:])

            # ---------- phase B: layer-1 recurrence ----------
            big = wp.tile([128, KC * G4], BF, tag="bigW")
            nc.sync.dma_start(big[:],
                              t_in["WhhT1"].ap().rearrange("(c p) n -> p c n", p=128))
            hT = None
            cst = [None] * NG
            for t in range(T):
                qb = sb3.tile([B, G4], BF, tag="qb")
                nc.sync.dma_start(qb[:], Q1[B * t:B * (t + 1), :])
                hT_next = sb.tile([128, 256], BF, tag="hTn")
                for n in range(NG):
                    g_ps = psg.tile([B, GW], F32, tag="g")
                    nc.tensor.matmul(g_ps[:], i32b[:],
                                     qb[:, GW * n:GW * (n + 1)],
                                     start=True, stop=(hT is None))
                    if hT is not None:
                        for c in range(KC):
                            nc.tensor.matmul(
                                g_ps[:], hT[:, 32 * c:32 * (c + 1)],
                                big[:, G4 * c + GW * n:G4 * c + GW * (n + 1)],
                                start=False, stop=(c == KC - 1))
                    cst[n] = cell_elem(g_ps, cst[n], hT_next, n)
                nc.scalar.dma_start(H1T[t], hT_next[:])
                hT = hT_next

            # ---------- fc ----------
            wfc = wp.tile([128, KC * M], BF, tag="wfc")
            nc.sync.dma_start(wfc[:],
                              t_in["WfcT"].ap().rearrange("(c p) m -> p c m", p=128))
            for m in range(T // 4):
                slab = sb3.tile([128, 1024], BF, tag="mslab", name=f"fslab{m}")
                nc.sync.dma_start(
                    slab[:],
                    H1T.ap()[4 * m:4 * (m + 1)].rearrange(
                        "t p (c b) -> p c t b", c=8))
                pps = psg.tile([128, M], F32, tag="g")
                nc.tensor.matmul(pps[:], ones128b[:], bfcb[:], start=True, stop=False)
                for c in range(KC):
                    nc.tensor.matmul(pps[:], slab[:, 128 * c:128 * (c + 1)],
                                     wfc[:, M * c:M * (c + 1)],
                                     start=False, stop=(c == KC - 1))
                pout = sb3.tile([128, M], F32, tag="pout")
                nc.scalar.activation(pout[:], pps[:], AF.Copy)
                for tt in range(4):
                    nc.sync.dma_start(out[:, 4 * m + tt, :],
                                      pout[32 * tt:32 * (tt + 1), :])
    nc.finalize()
    return nc, names


_CACHE = {}


def kernel(**inputs):
    import numpy as np
    from concourse.bass_utils import run_bass_kernel_spmd

    T = int(np.asarray(inputs["audio_targets"]).shape[1])
    if T not in _CACHE:
        _CACHE[T] = build(T)
    nc, _names = _CACHE[T]
    d = prep_inputs(inputs, T)
    n_cores = 8
    in_maps = [dict(d) for _ in range(n_cores)]
    res = run_bass_kernel_spmd(nc, in_maps, list(range(n_cores)))
    return np.asarray(res.results[0]["out"], dtype=np.float32)
